# revision 1
# baseline (speedup 1.0000x reference)
"""AttnBlock (GroupNorm -> QKV 1x1 -> spatial attention -> proj_out -> residual)
for Trainium2, sharded over 8 NeuronCores.

Sharding: (batch b in {0,1}) x (4 query chunks of 1024 of the 4096 spatial
positions). Every core runs the same program; per-core inputs are column-
rotated so the core's query block sits at columns 0..1023 (attention is
permutation-invariant over key positions, GroupNorm stats over column order).

Layouts avoid all on-chip transposes:
  K/Q:   [channel partitions, position free]
  V, P:  [position partitions, channel/query free]  (V^T = x^T @ Wv'^T via
         matmul with x as the stationary operand)
The GroupNorm affine is folded into the projection weights at runtime
(W' = W . diag(scale), bias' = W . shift + b), so projections consume raw x
chunks straight off DMA and no normalized copy of x is ever materialized.
Softmax row sums come from a ones-vector matmul; normalization is applied
after the P@V matmul (divides [512, 1024] instead of [4096, 1024]).
Matmuls run as float32r (full-rate fp32 path); V/P are stored bf16 for SBUF.
"""

import sys

sys.path.insert(0, "/opt/trn_rl_repo")

import numpy as np

C = 512
N = 4096  # h*w
NP = 4  # channel tiles of 128
QCH = 1024  # queries per core
EPS = 1e-6
GSIZE = 16  # channels per group
GELEMS = float(GSIZE * N)  # elements per group

_NC_CACHE = {}


def _build_nc(reps=1):
    import concourse.bacc as bacc
    import concourse.tile as tile
    from concourse import mybir

    dt = mybir.dt
    f32 = dt.float32
    f32r = dt.float32r
    bf16 = dt.bfloat16

    nc = bacc.Bacc("TRN2", target_bir_lowering=False, debug=False, num_devices=8)

    xb = nc.dram_tensor("xb", [C, N], f32r, kind="ExternalInput").ap()
    wkT = nc.dram_tensor("wkT", [C, C], f32r, kind="ExternalInput").ap()
    wvT = nc.dram_tensor("wvT", [C, C], f32r, kind="ExternalInput").ap()
    wqTs = nc.dram_tensor("wqTs", [C, C], f32r, kind="ExternalInput").ap()
    woT = nc.dram_tensor("woT", [C, C], f32r, kind="ExternalInput").ap()
    bk_col = nc.dram_tensor("bk_col", [C, 1], f32, kind="ExternalInput").ap()
    bv_row = nc.dram_tensor("bv_row", [1, C], f32, kind="ExternalInput").ap()
    bqs_col = nc.dram_tensor("bqs_col", [C, 1], f32, kind="ExternalInput").ap()
    bo_col = nc.dram_tensor("bo_col", [C, 1], f32, kind="ExternalInput").ap()
    gamma_col = nc.dram_tensor("gamma_col", [C, 1], f32, kind="ExternalInput").ap()
    beta_col = nc.dram_tensor("beta_col", [C, 1], f32, kind="ExternalInput").ap()
    g8 = nc.dram_tensor("g8", [128, 8], f32, kind="ExternalInput").ap()
    e8 = nc.dram_tensor("e8", [8, 128], f32, kind="ExternalInput").ap()
    y = nc.dram_tensor("y", [C, QCH], f32, kind="ExternalOutput").ap()

    with tile.TileContext(nc) as tc:
        with (
            tc.tile_pool(name="kpool", bufs=1) as kpool,      # K: 4 x [128,4096] f32r
            tc.tile_pool(name="vpool", bufs=1) as vpool,      # V^T: 32 x [128,512] bf16
            tc.tile_pool(name="qpool", bufs=1) as qpool,      # Q: 4 x [128,1024] f32r
            tc.tile_pool(name="wkv", bufs=1) as wkv,          # wkT+wvT tiles
            tc.tile_pool(name="wx", bufs=4) as wx,            # wqTs then woT (shared slots)
            tc.tile_pool(name="xc", bufs=8) as xcp,           # streamed x chunks [128,512]
            tc.tile_pool(name="scr", bufs=2) as scr,          # square scratch
            tc.tile_pool(name="pt", bufs=6) as ptp,           # exp(P) tiles bf16
            tc.tile_pool(name="att", bufs=4) as attp,         # attn output sbuf
            tc.tile_pool(name="ysb", bufs=4) as ysbp,         # final out tiles
            tc.tile_pool(name="xq", bufs=4) as xqp,           # residual chunks
            tc.tile_pool(name="small", bufs=1) as small,      # stats/bias vectors
            tc.tile_pool(name="ps", bufs=3, space="PSUM") as ps,
            tc.tile_pool(name="pv", bufs=4, space="PSUM") as pvp,
            tc.tile_pool(name="rs", bufs=1, space="PSUM") as rsp,
        ):
            # ---- persistent small tensors ----
            wk_t = [wkv.tile([128, C], f32r, tag=f"wk{p}", name=f"wk_t{p}") for p in range(NP)]
            wv_t = [wkv.tile([128, C], f32r, tag=f"wv{p}", name=f"wv_t{p}") for p in range(NP)]
            bk_t = [small.tile([128, 1], f32, tag=f"bk{p}", name=f"bk{p}") for p in range(NP)]
            bqs_t = [small.tile([128, 1], f32, tag=f"bqs{p}", name=f"bqs{p}") for p in range(NP)]
            bo_t = [small.tile([128, 1], f32, tag=f"bo{p}", name=f"bo{p}") for p in range(NP)]
            gam_t = [small.tile([128, 1], f32, tag=f"gam{p}", name=f"gam{p}") for p in range(NP)]
            bet_t = [small.tile([128, 1], f32, tag=f"bet{p}", name=f"bet{p}") for p in range(NP)]
            for p in range(NP):
                sl = slice(p * 128, (p + 1) * 128)
                nc.sync.dma_start(bk_t[p][:], bk_col[sl, :])
                nc.sync.dma_start(bqs_t[p][:], bqs_col[sl, :])
                nc.sync.dma_start(bo_t[p][:], bo_col[sl, :])
                nc.sync.dma_start(gam_t[p][:], gamma_col[sl, :])
                nc.sync.dma_start(bet_t[p][:], beta_col[sl, :])
            bv_t = small.tile([1, C], f32, tag="bv")
            nc.sync.dma_start(bv_t[:], bv_row[:])
            g8_t = small.tile([128, 8], f32, tag="g8")
            nc.sync.dma_start(g8_t[:], g8[:])
            e8_t = small.tile([8, 128], f32, tag="e8")
            nc.sync.dma_start(e8_t[:], e8[:])
            ones_t = small.tile([128, 1], bf16, tag="ones")
            nc.vector.memset(ones_t[:], 1.0)

            for _rep in range(reps):
                # ================= GroupNorm statistics =================
                # per-channel sum / sum-of-squares, streamed in [128,512]
                # chunks, DMA alternating between the HWDGE and SWDGE queues
                if _rep == 0:
                    for p in range(NP):
                        nc.gpsimd.dma_start(wk_t[p][:], wkT[p * 128:(p + 1) * 128, :])
                        nc.gpsimd.dma_start(wv_t[p][:], wvT[p * 128:(p + 1) * 128, :])
                wq_t = [wx.tile([128, C], f32r, tag="wx", name="wx_t") for _ in range(NP)]
                for p in range(NP):
                    nc.gpsimd.dma_start(wq_t[p][:], wqTs[p * 128:(p + 1) * 128, :])
                scale_t = [small.tile([128, 1], f32, tag=f"scale{p}", name=f"scale{p}") for p in range(NP)]
                shift_t = [small.tile([128, 1], f32, tag=f"shift{p}", name=f"shift{p}") for p in range(NP)]
                u_t = [small.tile([128, 1], f32, tag=f"u{p}", name=f"u{p}") for p in range(NP)]
                stc1 = [small.tile([128, 8], f32, tag=f"stc1_{p}", name=f"stc1_{p}") for p in range(NP)]
                stc2 = [small.tile([128, 8], f32, tag=f"stc2_{p}", name=f"stc2_{p}") for p in range(NP)]
                st = [small.tile([128, 2], f32, tag=f"st{p}", name=f"st{p}") for p in range(NP)]
                for p in range(NP):
                    for jb in range(8):
                        xt = xcp.tile([128, 512], f32r, tag="xchunk")
                        eng = nc.sync
                        eng.dma_start(
                            xt[:], xb[p * 128:(p + 1) * 128, jb * 512:(jb + 1) * 512]
                        )
                        nc.vector.reduce_sum(
                            stc1[p][:, jb:jb + 1], xt[:], axis=mybir.AxisListType.X
                        )
                        sq = scr.tile([128, 512], f32, tag="sq")
                        nc.scalar.activation(
                            sq[:], xt[:], mybir.ActivationFunctionType.Square,
                            accum_out=stc2[p][:, jb:jb + 1],
                        )
                    nc.vector.reduce_sum(
                        st[p][:, 0:1], stc1[p][:], axis=mybir.AxisListType.X
                    )
                    nc.vector.reduce_sum(
                        st[p][:, 1:2], stc2[p][:], axis=mybir.AxisListType.X
                    )
                    # tile p's group-stat chain + weight scaling, emitted here
                    # so it executes under tile p+1's stats DMA stream
                    pg = ps.tile([8, 2], f32, tag="pp", name="pg")
                    nc.tensor.matmul(pg[:], g8_t[:], st[p][:], start=True, stop=True)
                    pgs = small.tile([8, 2], f32, tag=f"pgs{p}", name=f"pgs{p}")
                    nc.vector.tensor_copy(pgs[:], pg[:])
                    gt = small.tile([8, 4], f32, tag=f"gt{p}", name=f"gt{p}")
                    nc.vector.tensor_mul(gt[:, 0:1], pgs[:, 0:1], pgs[:, 0:1])
                    nc.vector.tensor_sub(gt[:, 1:2], pgs[:, 1:2], gt[:, 0:1])
                    nc.vector.tensor_scalar_add(gt[:, 1:2], gt[:, 1:2], EPS)
                    nc.scalar.activation(
                        gt[:, 2:3], gt[:, 1:2], mybir.ActivationFunctionType.Sqrt
                    )
                    rb = small.tile([8, 2], f32, tag=f"rb{p}", name=f"rb{p}")
                    nc.vector.reciprocal(rb[:, 0:1], gt[:, 2:3])
                    nc.vector.tensor_mul(gt[:, 3:4], pgs[:, 0:1], rb[:, 0:1])
                    nc.vector.tensor_scalar_mul(rb[:, 1:2], gt[:, 3:4], -1.0)
                    pc = ps.tile([128, 2], f32, tag="pp", name="pc")
                    nc.tensor.matmul(pc[:], e8_t[:], rb[:], start=True, stop=True)
                    nc.vector.tensor_mul(scale_t[p][:], gam_t[p][:], pc[:, 0:1])
                    tsh = small.tile([128, 1], f32, tag=f"tsh{p}", name=f"tsh{p}")
                    nc.vector.tensor_mul(tsh[:], gam_t[p][:], pc[:, 1:2])
                    nc.vector.tensor_add(shift_t[p][:], bet_t[p][:], tsh[:])
                    nc.vector.tensor_scalar_mul(wk_t[p][:], wk_t[p][:], scale_t[p][:])
                    nc.vector.tensor_scalar_mul(wv_t[p][:], wv_t[p][:], scale_t[p][:])
                    nc.vector.tensor_scalar_mul(wq_t[p][:], wq_t[p][:], scale_t[p][:])
                    nc.vector.reciprocal(u_t[p][:], scale_t[p][:])
                    nc.vector.tensor_mul(u_t[p][:], u_t[p][:], shift_t[p][:])

                # ===== biases from scaled weights via u = shift/scale =====
                # (safe: scale = gamma*rstd and the model fixes gamma=1)
                biask_t = [small.tile([128, 1], f32, tag=f"biask{m}", name=f"biask{m}") for m in range(NP)]
                biasq_t = [small.tile([128, 1], f32, tag=f"biasq{m}", name=f"biasq{m}") for m in range(NP)]
                for m in range(NP):
                    msl = slice(m * 128, (m + 1) * 128)
                    pbk = ps.tile([128, 1], f32, tag="pp", name="pbk")
                    for p in range(NP):
                        nc.tensor.matmul(
                            pbk[:], wk_t[p][:, msl].bitcast(f32), u_t[p][:],
                            start=(p == 0), stop=(p == NP - 1),
                        )
                    nc.vector.tensor_add(biask_t[m][:], pbk[:], bk_t[m][:])
                    pbq = ps.tile([128, 1], f32, tag="pp", name="pbq")
                    for p in range(NP):
                        nc.tensor.matmul(
                            pbq[:], wq_t[p][:, msl].bitcast(f32), u_t[p][:],
                            start=(p == 0), stop=(p == NP - 1),
                        )
                    nc.vector.tensor_add(biasq_t[m][:], pbq[:], bqs_t[m][:])
                pbv = ps.tile([1, C], f32, tag="pp", name="pbv")
                for p in range(NP):
                    nc.tensor.matmul(
                        pbv[:], u_t[p][:], wv_t[p][:].bitcast(f32),
                        start=(p == 0), stop=(p == NP - 1),
                    )
                bvr = small.tile([1, C], f32, tag="bvr")
                nc.vector.tensor_add(bvr[:], pbv[:], bv_t[:])
                bv_bc = small.tile([128, C], f32, tag="bv_bc")
                nc.gpsimd.partition_broadcast(bv_bc[:], bvr[:])

                # ========== projections (streamed over 8 col chunks) ==========
                k_sb = [kpool.tile([128, N], f32r, tag=f"k{m}", name=f"k{m}") for m in range(NP)]
                vt_sb = [vpool.tile([128, C], bf16, tag=f"vt{j}", name=f"vt{j}") for j in range(32)]
                q_sb = [qpool.tile([128, QCH], f32r, tag=f"q{m}", name=f"q{m}") for m in range(NP)]

                for jb in range(8):
                    jsl = slice(jb * 512, (jb + 1) * 512)
                    x_c = []
                    for p in range(NP):
                        xt = xcp.tile([128, 512], f32r, tag="xchunk", name="xt2")
                        eng = nc.gpsimd if jb < 4 else nc.sync
                        eng.dma_start(xt[:], xb[p * 128:(p + 1) * 128, jsl])
                        x_c.append(xt)
                    # K chunk: [c_out tile m, 512 cols]
                    for m in range(NP):
                        msl = slice(m * 128, (m + 1) * 128)
                        pk = ps.tile([128, 512], f32, tag="pp", name="pk")
                        for p in range(NP):
                            nc.tensor.matmul(
                                pk[:], wk_t[p][:, msl], x_c[p][:],
                                start=(p == 0), stop=(p == NP - 1),
                            )
                        nc.scalar.activation(
                            k_sb[m][:, jsl], pk[:],
                            mybir.ActivationFunctionType.Identity,
                            bias=biask_t[m][:],
                        )
                    # V^T rows (4 j-tiles of 128): [j tile, c_out]
                    for jt in range(4):
                        jj = jb * 4 + jt
                        pvt = ps.tile([128, 512], f32, tag="pp", name="pvt")
                        for p in range(NP):
                            nc.tensor.matmul(
                                pvt[:], x_c[p][:, jt * 128:(jt + 1) * 128], wv_t[p][:],
                                start=(p == 0), stop=(p == NP - 1),
                            )
                        nc.vector.tensor_add(vt_sb[jj][:], pvt[:], bv_bc[:])
                    # Q (only first two chunks = this core's query block)
                    if jb < 2:
                        for m in range(NP):
                            msl = slice(m * 128, (m + 1) * 128)
                            pq = ps.tile([128, 512], f32, tag="pp", name="pq")
                            for p in range(NP):
                                nc.tensor.matmul(
                                    pq[:], wq_t[p][:, msl], x_c[p][:],
                                    start=(p == 0), stop=(p == NP - 1),
                                )
                            nc.scalar.activation(
                                q_sb[m][:, jsl], pq[:],
                                mybir.ActivationFunctionType.Identity,
                                bias=biasq_t[m][:],
                            )

                # ================= attention =================
                wo_t = [wx.tile([128, C], f32r, tag="wx", name="wx_t") for _ in range(NP)]
                for p in range(NP):
                    nc.sync.dma_start(wo_t[p][:], woT[p * 128:(p + 1) * 128, :])

                for ci in range(2):
                    isl = slice(ci * 512, (ci + 1) * 512)
                    pv_ps = [pvp.tile([128, 512], f32, tag="pv", name="pv_ps") for _ in range(NP)]
                    rs_ps = rsp.tile([1, 512], f32, tag="rs")
                    for jt in range(32):
                        st_ps = ps.tile([128, 512], f32, tag="pp", name="st_ps")
                        for p in range(NP):
                            nc.tensor.matmul(
                                st_ps[:],
                                k_sb[p][:, jt * 128:(jt + 1) * 128],
                                q_sb[p][:, isl],
                                start=(p == 0), stop=(p == NP - 1),
                            )
                        pt = ptp.tile([128, 512], bf16, tag="pt")
                        nc.scalar.activation(
                            pt[:], st_ps[:], mybir.ActivationFunctionType.Exp
                        )
                        nc.tensor.matmul(
                            rs_ps[:], ones_t[:], pt[:],
                            start=(jt == 0), stop=(jt == 31),
                        )
                        for m in range(NP):
                            nc.tensor.matmul(
                                pv_ps[m][:],
                                vt_sb[jt][:, m * 128:(m + 1) * 128],
                                pt[:],
                                start=(jt == 0), stop=(jt == 31),
                            )
                    recip = small.tile([1, 512], f32, tag="recip")
                    nc.vector.reciprocal(recip[:], rs_ps[:])
                    recip_bc = small.tile([128, 512], f32, tag="recip_bc")
                    nc.gpsimd.partition_broadcast(recip_bc[:], recip[:])
                    att = []
                    for m in range(NP):
                        a = attp.tile([128, 512], f32r, tag="att", name="att_t")
                        nc.vector.tensor_mul(a[:], pv_ps[m][:], recip_bc[:])
                        att.append(a)
                    # proj_out + bias + residual
                    for m in range(NP):
                        msl = slice(m * 128, (m + 1) * 128)
                        po = ps.tile([128, 512], f32, tag="pp", name="po")
                        for p in range(NP):
                            nc.tensor.matmul(
                                po[:], wo_t[p][:, msl], att[p][:],
                                start=(p == 0), stop=(p == NP - 1),
                            )
                        yt = ysbp.tile([128, 512], f32, tag="ysb")
                        nc.scalar.activation(
                            yt[:], po[:], mybir.ActivationFunctionType.Identity,
                            bias=bo_t[m][:],
                        )
                        xq_t = xqp.tile([128, 512], f32, tag="xq")
                        nc.sync.dma_start(xq_t[:], xb[msl, isl].bitcast(f32))
                        nc.vector.tensor_add(yt[:], yt[:], xq_t[:])
                        nc.sync.dma_start(y[msl, isl], yt[:])

    nc.compile()
    return nc


def get_nc(reps=1):
    if reps not in _NC_CACHE:
        _NC_CACHE[reps] = _build_nc(reps)
    return _NC_CACHE[reps]


def make_in_maps(x, gn_gamma, gn_beta, wq, bq, wk, bk, wv, bv, wo, bo):
    s = 1.0 / np.sqrt(C)
    shared = {
        "wkT": np.ascontiguousarray(np.asarray(wk).T).astype(np.float32),
        "wvT": np.ascontiguousarray(np.asarray(wv).T).astype(np.float32),
        "wqTs": (np.ascontiguousarray(np.asarray(wq).T) * s).astype(np.float32),
        "woT": np.ascontiguousarray(np.asarray(wo).T).astype(np.float32),
        "bk_col": np.ascontiguousarray(np.asarray(bk)[:, None]).astype(np.float32),
        "bv_row": np.ascontiguousarray(np.asarray(bv)[None, :]).astype(np.float32),
        "bqs_col": (np.asarray(bq)[:, None] * s).astype(np.float32),
        "bo_col": np.ascontiguousarray(np.asarray(bo)[:, None]).astype(np.float32),
        "gamma_col": np.ascontiguousarray(np.asarray(gn_gamma)[:, None]).astype(np.float32),
        "beta_col": np.ascontiguousarray(np.asarray(gn_beta)[:, None]).astype(np.float32),
    }
    g8 = np.zeros((128, 8), np.float32)
    for i in range(128):
        g8[i, i // GSIZE] = 1.0 / GELEMS
    e8 = np.zeros((8, 128), np.float32)
    for i in range(128):
        e8[i // GSIZE, i] = 1.0
    shared["g8"] = g8
    shared["e8"] = e8

    xf = np.asarray(x, np.float32).reshape(2, C, N)
    in_maps = []
    for cid in range(8):
        bi, qc = cid // 4, cid % 4
        xbv = np.ascontiguousarray(np.roll(xf[bi], -qc * QCH, axis=1))
        in_maps.append({"xb": xbv, **shared})
    return in_maps


def kernel(**inputs):
    from concourse.bass_utils import run_bass_kernel_spmd

    x = np.asarray(inputs["x"], np.float32)
    in_maps = make_in_maps(
        x, inputs["gn_gamma"], inputs["gn_beta"],
        inputs["wq"], inputs["bq"], inputs["wk"], inputs["bk"],
        inputs["wv"], inputs["bv"], inputs["wo"], inputs["bo"],
    )
    nc = get_nc(reps=1)
    res = run_bass_kernel_spmd(nc, in_maps, core_ids=list(range(8)), trace=False)
    out = np.empty((2, C, N), np.float32)
    for cid in range(8):
        bi, qc = cid // 4, cid % 4
        out[bi][:, qc * QCH:(qc + 1) * QCH] = res.results[cid]["y"]
    return out.reshape(2, C, 64, 64)


if __name__ == "__main__":
    rng = np.random.default_rng(0)
    inputs = {
        "x": rng.standard_normal((2, C, 64, 64), dtype=np.float32),
        "gn_gamma": np.ones(C, np.float32),
        "gn_beta": np.zeros(C, np.float32),
    }
    s = 1.0 / np.sqrt(C)
    for nm in ("q", "k", "v", "o"):
        inputs[f"w{nm}"] = (rng.standard_normal((C, C), dtype=np.float32) * s)
        inputs[f"b{nm}"] = (rng.standard_normal(C, dtype=np.float32) * 0.01)
    out = kernel(**inputs)
    print("kernel ran, out shape", out.shape, "mean", out.mean())



# revision 8
# speedup vs baseline: 1.6959x; 1.6959x over previous
"""AttnBlock (GroupNorm -> QKV 1x1 -> spatial attention -> proj_out -> residual)
for Trainium2, sharded over 8 NeuronCores.

Sharding: (batch b in {0,1}) x (4 query chunks of 1024 of the 4096 spatial
positions). Every core runs the same program; per-core inputs are column-
rotated so the core's query block sits at columns 0..1023.

fp8 (e4m3) DoubleRow design: all heavy matmuls run as fp8 DoubleRow pairs
(contraction 256 per instruction, 0.5 PE cycles per output row).
  x arrives pre-quantized fp8 [512, 4096]. GroupNorm stats are computed on
  chip (sum via GpSimd tensor_scalar+accum, sum-of-squares via DVE
  tensor_tensor_reduce), the affine normalize (scale*x+shift) produces fp8
  xn in DoubleRow pair layout. Weights arrive fp8, pre-transposed and
  prescaled by 16 (to keep them in e4m3 normal range); the 1/16 and the
  projection biases are folded into the mandatory PSUM->SBUF fp8 conversion
  passes (2-op tensor_scalar / scalar_tensor_tensor), which are spread
  across the DVE and GpSimd engines. The 1/sqrt(C) softmax scale and the
  exp range shift (-3) are folded into the Exp activation (scale/bias).
  Row sums come from a DoubleRow ones-matmul; normalization is applied to
  the [512, 1024] attention output, not the [4096, 1024] probabilities.
  Output and residual stream as bf16.
"""

import sys

sys.path.insert(0, "/opt/trn_rl_repo")

import numpy as np
import ml_dtypes

C = 512
N = 4096  # h*w
QCH = 1024  # queries per core
EPS = 1e-6
GSIZE = 16  # channels per group
GELEMS = float(GSIZE * N)  # elements per group
WPRE = 16.0  # weight prescale before fp8 quantization
E4 = ml_dtypes.float8_e4m3
BF16 = ml_dtypes.bfloat16

_NC_CACHE = {}


def _build_nc(reps=1):
    import concourse.bacc as bacc
    import concourse.tile as tile
    from concourse import mybir

    dt = mybir.dt
    f32 = dt.float32
    f8 = dt.float8e4
    bf16 = dt.bfloat16
    DR = mybir.MatmulPerfMode.DoubleRow
    MUL = mybir.AluOpType.mult
    ADD = mybir.AluOpType.add

    nc = bacc.Bacc("TRN2", target_bir_lowering=False, debug=False, num_devices=8)

    x8_d = nc.dram_tensor("x8", [C, N], f8, kind="ExternalInput").ap()
    xres_d = nc.dram_tensor("xres", [C, QCH], bf16, kind="ExternalInput").ap()
    wk_d = nc.dram_tensor("wk8", [256, 2, 512], f8, kind="ExternalInput").ap()
    wv_d = nc.dram_tensor("wv8", [256, 2, 512], f8, kind="ExternalInput").ap()
    wq_d = nc.dram_tensor("wq8", [256, 2, 512], f8, kind="ExternalInput").ap()
    wo_d = nc.dram_tensor("wo8", [256, 2, 512], f8, kind="ExternalInput").ap()
    bkp_d = nc.dram_tensor("bkp", [128, 4], f32, kind="ExternalInput").ap()
    bqp_d = nc.dram_tensor("bqp", [128, 4], f32, kind="ExternalInput").ap()
    bop_d = nc.dram_tensor("bop", [128, 4], f32, kind="ExternalInput").ap()
    gam_d = nc.dram_tensor("gam", [128, 4], f32, kind="ExternalInput").ap()
    bet_d = nc.dram_tensor("bet", [128, 4], f32, kind="ExternalInput").ap()
    g16_d = nc.dram_tensor("g16", [128, 8], f32, kind="ExternalInput").ap()
    e16_d = nc.dram_tensor("e16", [8, 128], f32, kind="ExternalInput").ap()
    y_d = nc.dram_tensor("y", [C, QCH], bf16, kind="ExternalOutput").ap()

    with tile.TileContext(nc) as tc:
        with (
            tc.tile_pool(name="wp", bufs=1) as wp,       # weights fp8
            tc.tile_pool(name="xp", bufs=1) as xp,       # x fp8 tiles
            tc.tile_pool(name="xn", bufs=1) as xnp_p,    # normalized x pairs
            tc.tile_pool(name="kp", bufs=1) as kp_p,     # K pairs
            tc.tile_pool(name="vp", bufs=1) as vp_p,     # V^T pairs
            tc.tile_pool(name="qp", bufs=1) as qp_p,     # Q pairs
            tc.tile_pool(name="pt", bufs=4) as pt_p,     # exp(P) pair tiles
            tc.tile_pool(name="at", bufs=4) as at_p,     # attn fp8 pairs
            tc.tile_pool(name="xr", bufs=1) as xr_p,     # residual bf16
            tc.tile_pool(name="yy", bufs=4) as y_p,      # out tiles bf16
            tc.tile_pool(name="sc", bufs=2) as scr_p,    # fp8 scratch
            tc.tile_pool(name="sm", bufs=1) as sm,       # small tensors
            tc.tile_pool(name="ps", bufs=3, space="PSUM") as ps,
            tc.tile_pool(name="pv", bufs=4, space="PSUM") as pvp,
            tc.tile_pool(name="rs", bufs=1, space="PSUM") as rsp,
        ):
            # ---- persistent small tensors ----
            bkp_t = sm.tile([128, 4], f32, tag="bkp")
            bqp_t = sm.tile([128, 4], f32, tag="bqp")
            bop_t = sm.tile([128, 4], f32, tag="bop")
            gam_t = sm.tile([128, 4], f32, tag="gam")
            bet_t = sm.tile([128, 4], f32, tag="bet")
            g16_t = sm.tile([128, 8], f32, tag="g16")
            e16_t = sm.tile([8, 128], f32, tag="e16")
            for t, d in ((bkp_t, bkp_d), (bqp_t, bqp_d), (bop_t, bop_d),
                         (gam_t, gam_d), (bet_t, bet_d), (g16_t, g16_d),
                         (e16_t, e16_d)):
                nc.sync.dma_start(t[:], d[:])
            zero_c = sm.tile([128, 1], f32, tag="zero_c")
            nc.vector.memset(zero_c[:], 0.0)
            ones16 = sm.tile([128, 2, 16], f8, tag="ones16")
            nc.vector.memset(ones16[:], 0.0)
            nc.vector.memset(ones16[:, :, 0:1], 1.0)
            neg3 = sm.tile([128, 1], f32, tag="neg3")
            nc.vector.memset(neg3[:], -3.0)

            # weights
            wk_t = [wp.tile([128, 2, 512], f8, tag=f"wk{p}", name=f"wk{p}") for p in range(2)]
            wv_t = [wp.tile([128, 2, 512], f8, tag=f"wv{p}", name=f"wv{p}") for p in range(2)]
            wq_t = [wp.tile([128, 2, 512], f8, tag=f"wq{p}", name=f"wq{p}") for p in range(2)]
            wo_t = [wp.tile([128, 2, 512], f8, tag=f"wo{p}", name=f"wo{p}") for p in range(2)]
            for p in range(2):
                sl = slice(p * 128, (p + 1) * 128)
                nc.sync.dma_start(wk_t[p][:], wk_d[sl, :, :])
                nc.sync.dma_start(wv_t[p][:], wv_d[sl, :, :])
                nc.sync.dma_start(wq_t[p][:], wq_d[sl, :, :])
                nc.sync.dma_start(wo_t[p][:], wo_d[sl, :, :])
            xr_t = [xr_p.tile([128, QCH], bf16, tag=f"xr{m}", name=f"xr{m}") for m in range(4)]
            for m in range(4):
                nc.sync.dma_start(xr_t[m][:], xres_d[m * 128:(m + 1) * 128, :])

            for _rep in range(reps):
                # ================= GroupNorm statistics =================
                x8_t = [xp.tile([128, N], f8, tag=f"x{p}", name=f"x{p}")
                        for p in range(4)]
                for p in range(4):
                    nc.sync.dma_start(x8_t[p][:], x8_d[p * 128:(p + 1) * 128, :])
                st2 = sm.tile([128, 8], f32, tag="st2", name="st2")
                for p in range(4):
                    nc.vector.tensor_reduce(
                        st2[:, 2 * p:2 * p + 1], x8_t[p][:],
                        mybir.AxisListType.X, ADD,
                    )
                    scr_v = scr_p.tile([128, N], f8, tag="scrv", name="scrv")
                    nc.scalar.activation(
                        scr_v[:], x8_t[p][:],
                        mybir.ActivationFunctionType.Square,
                        accum_out=st2[:, 2 * p + 1:2 * p + 2],
                    )
                # group stats: [Sx, Sxx] per channel -> per group (16 ch)
                pg_ps = ps.tile([8, 8], f32, tag="pp", name="pg")
                nc.tensor.matmul(pg_ps[:], g16_t[:], st2[:], start=True, stop=True)
                pg = sm.tile([8, 8], f32, tag="pg", name="pg_sb")
                nc.vector.tensor_copy(pg[:], pg_ps[:])
                # mu = pg[:, 0::2], E2 = pg[:, 1::2]  (both scaled by 1/GELEMS)
                mu = pg[:].rearrange("p (f two) -> p f two", two=2)[:, :, 0:1].squeeze(2)
                e2 = pg[:].rearrange("p (f two) -> p f two", two=2)[:, :, 1:2].squeeze(2)
                var = sm.tile([8, 4], f32, tag="var", name="var")
                nc.vector.tensor_tensor(var[:], mu, mu, MUL)
                nc.vector.tensor_sub(var[:], e2, var[:])
                nc.vector.tensor_scalar_add(var[:], var[:], EPS)
                sd = sm.tile([8, 4], f32, tag="sd", name="sd")
                nc.scalar.activation(sd[:], var[:],
                                     mybir.ActivationFunctionType.Sqrt)
                rb = sm.tile([8, 8], f32, tag="rb", name="rb")
                rstd = rb[:].rearrange("p (f two) -> p f two", two=2)[:, :, 0:1].squeeze(2)
                nmr = rb[:].rearrange("p (f two) -> p f two", two=2)[:, :, 1:2].squeeze(2)
                nc.vector.reciprocal(rstd, sd[:])
                nc.vector.tensor_tensor(nmr, mu, rstd, MUL)
                nc.vector.tensor_scalar_mul(nmr, nmr, -1.0)
                pc_ps = ps.tile([128, 8], f32, tag="pp", name="pc")
                nc.tensor.matmul(pc_ps[:], e16_t[:], rb[:], start=True, stop=True)
                pc = sm.tile([128, 8], f32, tag="pc", name="pc_sb")
                nc.vector.tensor_copy(pc[:], pc_ps[:])
                pc_r = pc[:].rearrange("p (f two) -> p f two", two=2)
                scale_a = sm.tile([128, 4], f32, tag="scale", name="scale")
                shift_a = sm.tile([128, 4], f32, tag="shift", name="shift")
                nc.vector.tensor_tensor(scale_a[:], gam_t[:], pc_r[:, :, 0:1].squeeze(2), MUL)
                nc.vector.tensor_tensor(shift_a[:], gam_t[:], pc_r[:, :, 1:2].squeeze(2), MUL)
                nc.vector.tensor_add(shift_a[:], shift_a[:], bet_t[:])

                # ================= normalize -> fp8 pairs =================
                xnp = [xnp_p.tile([128, 2, N], f8, tag=f"xn{pp}", name=f"xn{pp}")
                       for pp in range(2)]
                for p in range(4):
                    nc.gpsimd.tensor_scalar(
                        xnp[p // 2][:, p % 2, :], x8_t[p][:],
                        scale_a[:, p:p + 1], shift_a[:, p:p + 1], MUL, ADD,
                    )

                # ================= projections =================
                kp = [kp_p.tile([128, 2, N], f8, tag=f"k{pp}", name=f"k{pp}")
                      for pp in range(2)]
                vtp = [vp_p.tile([128, 2, 512], f8, tag=f"v{t}", name=f"v{t}")
                       for t in range(16)]
                qp = [qp_p.tile([128, 2, QCH], f8, tag=f"q{pp}", name=f"q{pp}")
                      for pp in range(2)]

                for jb in range(8):
                    jsl = slice(jb * 512, (jb + 1) * 512)
                    # K: out[c_out m, j]; conversions on gpsimd/DVE alternating
                    for m in range(4):
                        pk = ps.tile([128, 512], f32, tag="pp", name="pk")
                        for pp in range(2):
                            nc.tensor.matmul(
                                pk[:], wk_t[pp][:, :, m * 128:(m + 1) * 128],
                                xnp[pp][:, :, jsl],
                                start=(pp == 0), stop=(pp == 1), perf_mode=DR,
                            )
                        if m % 2 == 0:
                            nc.scalar.activation(
                                kp[m // 2][:, m % 2, jsl], pk[:],
                                mybir.ActivationFunctionType.Identity,
                                bias=bkp_t[:, m:m + 1], scale=1.0 / WPRE,
                            )
                        else:
                            nc.vector.tensor_scalar(
                                kp[m // 2][:, m % 2, jsl], pk[:],
                                1.0 / WPRE, bkp_t[:, m:m + 1], MUL, ADD,
                            )
                    # V^T: out[j, c_out] per jt
                    for jt4 in range(4):
                        jt = jb * 4 + jt4
                        pv_ = ps.tile([128, 512], f32, tag="pp", name="pvt")
                        for pp in range(2):
                            nc.tensor.matmul(
                                pv_[:], xnp[pp][:, :, jt * 128:(jt + 1) * 128],
                                wv_t[pp][:],
                                start=(pp == 0), stop=(pp == 1), perf_mode=DR,
                            )
                        if jt4 % 2 == 0:
                            nc.scalar.activation(
                                vtp[jt // 2][:, jt % 2, :], pv_[:],
                                mybir.ActivationFunctionType.Identity,
                                bias=zero_c[:], scale=1.0 / WPRE,
                            )
                        else:
                            nc.vector.tensor_scalar(
                                vtp[jt // 2][:, jt % 2, :], pv_[:],
                                1.0 / WPRE, None, MUL,
                            )
                    # Q (first two chunks only)
                    if jb < 2:
                        for m in range(4):
                            pq = ps.tile([128, 512], f32, tag="pp", name="pq")
                            for pp in range(2):
                                nc.tensor.matmul(
                                    pq[:], wq_t[pp][:, :, m * 128:(m + 1) * 128],
                                    xnp[pp][:, :, jsl],
                                    start=(pp == 0), stop=(pp == 1), perf_mode=DR,
                                )
                            if m % 2 == 0:
                                nc.scalar.activation(
                                    qp[m // 2][:, m % 2, jsl], pq[:],
                                    mybir.ActivationFunctionType.Identity,
                                    bias=bqp_t[:, m:m + 1], scale=1.0 / WPRE,
                                )
                            else:
                                nc.vector.tensor_scalar(
                                    qp[m // 2][:, m % 2, jsl], pq[:],
                                    1.0 / WPRE, bqp_t[:, m:m + 1], MUL, ADD,
                                )

                # ================= attention =================
                SSC = 1.0 / np.sqrt(C)  # softmax scale (k,q already natural scale)
                for ci in range(2):
                    isl = slice(ci * 512, (ci + 1) * 512)
                    pv_ps = [pvp.tile([128, 512], f32, tag="pv", name="pv_ps")
                             for _ in range(4)]
                    rs_ps = rsp.tile([16, 512], f32, tag="rs")
                    ptt = None
                    for jt in range(32):
                        st_ps = ps.tile([128, 512], f32, tag="pp", name="st")
                        for pp in range(2):
                            nc.tensor.matmul(
                                st_ps[:],
                                kp[pp][:, :, jt * 128:(jt + 1) * 128],
                                qp[pp][:, :, isl],
                                start=(pp == 0), stop=(pp == 1), perf_mode=DR,
                            )
                        if jt % 2 == 0:
                            ptt = pt_p.tile([128, 2, 512], f8, tag="pt", name="pt")
                        nc.scalar.activation(
                            ptt[:, jt % 2, :], st_ps[:],
                            mybir.ActivationFunctionType.Exp,
                            bias=neg3[:], scale=SSC,
                        )
                        if jt % 2 == 1:
                            t = jt // 2
                            nc.tensor.matmul(
                                rs_ps[:], ones16[:], ptt[:],
                                start=(t == 0), stop=(t == 15), perf_mode=DR,
                            )
                            for m in range(4):
                                nc.tensor.matmul(
                                    pv_ps[m][:],
                                    vtp[t][:, :, m * 128:(m + 1) * 128],
                                    ptt[:],
                                    start=(t == 0), stop=(t == 15), perf_mode=DR,
                                )
                    recip = sm.tile([1, 512], f32, tag="recip", name="recip")
                    nc.vector.reciprocal(recip[:], rs_ps[0:1, :])
                    recip_bc = sm.tile([128, 512], f32, tag="recip_bc", name="recip_bc")
                    nc.gpsimd.partition_broadcast(recip_bc[:], recip[:])
                    attp = [at_p.tile([128, 2, 512], f8, tag="att", name="att")
                            for _ in range(2)]
                    for m in range(4):
                        nc.vector.tensor_tensor(
                            attp[m // 2][:, m % 2, :], pv_ps[m][:], recip_bc[:], MUL,
                        )
                    # proj_out + bias + residual
                    for m in range(4):
                        po = ps.tile([128, 512], f32, tag="pp", name="po")
                        for pp in range(2):
                            nc.tensor.matmul(
                                po[:], wo_t[pp][:, :, m * 128:(m + 1) * 128],
                                attp[pp][:],
                                start=(pp == 0), stop=(pp == 1), perf_mode=DR,
                            )
                        yt = y_p.tile([128, 512], bf16, tag="y", name="yt")
                        poo = sm.tile([128, 512], f32, tag=f"poo{m}", name="poo")
                        nc.vector.tensor_scalar(
                            poo[:], po[:], 1.0 / WPRE, bop_t[:, m:m + 1], MUL, ADD,
                        )
                        nc.vector.tensor_tensor(yt[:], poo[:], xr_t[m][:, isl], ADD)
                        nc.sync.dma_start(y_d[m * 128:(m + 1) * 128, isl], yt[:])

    nc.compile()
    return nc


def get_nc(reps=1):
    if reps not in _NC_CACHE:
        _NC_CACHE[reps] = _build_nc(reps)
    return _NC_CACHE[reps]


def _pack_weight(w, prescale):
    # w: [c_out, c_in] -> wT [c_in, c_out] -> [pp*128+p, t, c_out]
    wT = np.ascontiguousarray(np.asarray(w, np.float32).T) * prescale
    arr = wT.reshape(2, 2, 128, C).transpose(0, 2, 1, 3).reshape(256, 2, C)
    return np.ascontiguousarray(arr).astype(E4)


def make_in_maps(x, gn_gamma, gn_beta, wq, bq, wk, bk, wv, bv, wo, bo):
    shared = {
        "wk8": _pack_weight(wk, WPRE),
        "wv8": _pack_weight(wv, WPRE),
        "wq8": _pack_weight(wq, WPRE),
        "wo8": _pack_weight(wo, WPRE),
        "bkp": np.ascontiguousarray(np.asarray(bk, np.float32).reshape(4, 128).T),
        "bqp": np.ascontiguousarray(np.asarray(bq, np.float32).reshape(4, 128).T),
        "bop": np.ascontiguousarray(
            (np.asarray(bo, np.float32)
             + np.asarray(wo, np.float32) @ np.asarray(bv, np.float32)
             ).reshape(4, 128).T),
        "gam": np.ascontiguousarray(np.asarray(gn_gamma, np.float32).reshape(4, 128).T),
        "bet": np.ascontiguousarray(np.asarray(gn_beta, np.float32).reshape(4, 128).T),
    }
    g16 = np.zeros((128, 8), np.float32)
    for i in range(128):
        g16[i, i // GSIZE] = 1.0 / GELEMS
    e16 = np.zeros((8, 128), np.float32)
    for i in range(128):
        e16[i // GSIZE, i] = 1.0
    shared["g16"] = g16
    shared["e16"] = e16

    xf = np.asarray(x, np.float32).reshape(2, C, N)
    in_maps = []
    for cid in range(8):
        bi, qc = cid // 4, cid % 4
        xr = np.ascontiguousarray(np.roll(xf[bi], -qc * QCH, axis=1))
        in_maps.append({
            "x8": xr.astype(E4),
            "xres": xr[:, :QCH].astype(BF16),
            **shared,
        })
    return in_maps


def kernel(**inputs):
    from concourse.bass_utils import run_bass_kernel_spmd

    x = np.asarray(inputs["x"], np.float32)
    in_maps = make_in_maps(
        x, inputs["gn_gamma"], inputs["gn_beta"],
        inputs["wq"], inputs["bq"], inputs["wk"], inputs["bk"],
        inputs["wv"], inputs["bv"], inputs["wo"], inputs["bo"],
    )
    nc = get_nc(reps=1)
    res = run_bass_kernel_spmd(nc, in_maps, core_ids=list(range(8)), trace=False)
    out = np.empty((2, C, N), np.float32)
    for cid in range(8):
        bi, qc = cid // 4, cid % 4
        out[bi][:, qc * QCH:(qc + 1) * QCH] = np.asarray(
            res.results[cid]["y"]).astype(np.float32)
    return out.reshape(2, C, 64, 64)


if __name__ == "__main__":
    rng = np.random.default_rng(0)
    inputs = {
        "x": rng.standard_normal((2, C, 64, 64), dtype=np.float32),
        "gn_gamma": np.ones(C, np.float32),
        "gn_beta": np.zeros(C, np.float32),
    }
    s = 1.0 / np.sqrt(C)
    for nm in ("q", "k", "v", "o"):
        inputs[f"w{nm}"] = (rng.standard_normal((C, C), dtype=np.float32) * s)
        inputs[f"b{nm}"] = (rng.standard_normal(C, dtype=np.float32) * 0.01)
    out = kernel(**inputs)
    print("kernel ran, out shape", out.shape, "mean", out.mean())


# revision 10
# speedup vs baseline: 2.1314x; 1.2568x over previous
"""AttnBlock (GroupNorm -> QKV 1x1 -> spatial attention -> proj_out -> residual)
for Trainium2, sharded over 8 NeuronCores.

Sharding: (batch b in {0,1}) x (4 query chunks of 1024 of the 4096 spatial
positions). Every core runs the same program; per-core inputs are column-
rotated so the core's query block sits at columns 0..1023.

fp8 (e4m3) DoubleRow design: all heavy matmuls run as fp8 DoubleRow pairs
(contraction 256 per instruction, 0.5 PE cycles per output row).
  x arrives pre-quantized fp8 [512, 4096]. GroupNorm stats are computed on
  chip (sum via GpSimd tensor_scalar+accum, sum-of-squares via DVE
  tensor_tensor_reduce), the affine normalize (scale*x+shift) produces fp8
  xn in DoubleRow pair layout. Weights arrive fp8, pre-transposed and
  prescaled by 16 (to keep them in e4m3 normal range); the 1/16 and the
  projection biases are folded into the mandatory PSUM->SBUF fp8 conversion
  passes (2-op tensor_scalar / scalar_tensor_tensor), which are spread
  across the DVE and GpSimd engines. The 1/sqrt(C) softmax scale and the
  exp range shift (-3) are folded into the Exp activation (scale/bias).
  Row sums come from a DoubleRow ones-matmul; normalization is applied to
  the [512, 1024] attention output, not the [4096, 1024] probabilities.
  Output and residual stream as bf16.
"""

import sys

sys.path.insert(0, "/opt/trn_rl_repo")

import numpy as np
import ml_dtypes

C = 512
N = 4096  # h*w
QCH = 1024  # queries per core
EPS = 1e-6
GSIZE = 16  # channels per group
GELEMS = float(GSIZE * N)  # elements per group
WPRE = 16.0  # weight prescale before fp8 quantization
E4 = ml_dtypes.float8_e4m3
BF16 = ml_dtypes.bfloat16

_NC_CACHE = {}


def _build_nc(reps=1):
    import concourse.bacc as bacc
    import concourse.tile as tile
    from concourse import mybir

    dt = mybir.dt
    f32 = dt.float32
    f8 = dt.float8e4
    bf16 = dt.bfloat16
    DR = mybir.MatmulPerfMode.DoubleRow
    MUL = mybir.AluOpType.mult
    ADD = mybir.AluOpType.add

    nc = bacc.Bacc("TRN2", target_bir_lowering=False, debug=False, num_devices=8)

    x8_d = nc.dram_tensor("x8", [C, N], f8, kind="ExternalInput").ap()
    xres_d = nc.dram_tensor("xres", [C, QCH], bf16, kind="ExternalInput").ap()
    wk_d = nc.dram_tensor("wk8", [256, 2, 512], f8, kind="ExternalInput").ap()
    wv_d = nc.dram_tensor("wv8", [256, 2, 512], f8, kind="ExternalInput").ap()
    wq_d = nc.dram_tensor("wq8", [256, 2, 512], f8, kind="ExternalInput").ap()
    wo_d = nc.dram_tensor("wo8", [256, 2, 512], f8, kind="ExternalInput").ap()
    bkp_d = nc.dram_tensor("bkp", [128, 4], f32, kind="ExternalInput").ap()
    bqp_d = nc.dram_tensor("bqp", [128, 4], f32, kind="ExternalInput").ap()
    gam_d = nc.dram_tensor("gam", [128, 4], f32, kind="ExternalInput").ap()
    bet_d = nc.dram_tensor("bet", [128, 4], f32, kind="ExternalInput").ap()
    g16_d = nc.dram_tensor("g16", [128, 8], f32, kind="ExternalInput").ap()
    e16_d = nc.dram_tensor("e16", [8, 128], f32, kind="ExternalInput").ap()
    y_d = nc.dram_tensor("y", [C, QCH], bf16, kind="ExternalOutput").ap()

    with tile.TileContext(nc) as tc:
        with (
            tc.tile_pool(name="wp", bufs=1) as wp,       # weights fp8
            tc.tile_pool(name="xp", bufs=1) as xp,       # x fp8 tiles
            tc.tile_pool(name="xn", bufs=1) as xnp_p,    # normalized x pairs
            tc.tile_pool(name="kp", bufs=1) as kp_p,     # K pairs
            tc.tile_pool(name="vp", bufs=1) as vp_p,     # V^T pairs
            tc.tile_pool(name="qp", bufs=1) as qp_p,     # Q pairs
            tc.tile_pool(name="pt", bufs=4) as pt_p,     # exp(P) pair tiles
            tc.tile_pool(name="at", bufs=4) as at_p,     # attn fp8 pairs
            tc.tile_pool(name="xr", bufs=1) as xr_p,     # residual bf16
            tc.tile_pool(name="yy", bufs=4) as y_p,      # out tiles bf16
            tc.tile_pool(name="sc", bufs=2) as scr_p,    # fp8 scratch
            tc.tile_pool(name="sm", bufs=1) as sm,       # small tensors
            tc.tile_pool(name="ps", bufs=3, space="PSUM") as ps,
            tc.tile_pool(name="pv", bufs=4, space="PSUM") as pvp,
            tc.tile_pool(name="rs", bufs=1, space="PSUM") as rsp,
        ):
            # ---- persistent small tensors ----
            bkp_t = sm.tile([128, 4], f32, tag="bkp")
            bqp_t = sm.tile([128, 4], f32, tag="bqp")
            gam_t = sm.tile([128, 4], f32, tag="gam")
            bet_t = sm.tile([128, 4], f32, tag="bet")
            g16_t = sm.tile([128, 8], f32, tag="g16")
            e16_t = sm.tile([8, 128], f32, tag="e16")
            for t, d in ((bkp_t, bkp_d), (bqp_t, bqp_d),
                         (gam_t, gam_d), (bet_t, bet_d), (g16_t, g16_d),
                         (e16_t, e16_d)):
                nc.gpsimd.dma_start(t[:], d[:])
            zero_c = sm.tile([128, 1], f32, tag="zero_c")
            nc.vector.memset(zero_c[:], 0.0)
            ones16 = sm.tile([128, 2, 16], f8, tag="ones16")
            nc.vector.memset(ones16[:], 0.0)
            nc.vector.memset(ones16[:, :, 0:1], 1.0)
            neg3 = sm.tile([128, 1], f32, tag="neg3")
            nc.vector.memset(neg3[:], -3.0)

            # weights
            wk_t = [wp.tile([128, 2, 512], f8, tag=f"wk{p}", name=f"wk{p}") for p in range(2)]
            wv_t = [wp.tile([128, 2, 512], f8, tag=f"wv{p}", name=f"wv{p}") for p in range(2)]
            wq_t = [wp.tile([128, 2, 512], f8, tag=f"wq{p}", name=f"wq{p}") for p in range(2)]
            wo_t = [wp.tile([128, 2, 512], f8, tag=f"wo{p}", name=f"wo{p}") for p in range(2)]
            for p in range(2):
                sl = slice(p * 128, (p + 1) * 128)
                nc.gpsimd.dma_start(wk_t[p][:], wk_d[sl, :, :])
                nc.gpsimd.dma_start(wv_t[p][:], wv_d[sl, :, :])
                nc.gpsimd.dma_start(wq_t[p][:], wq_d[sl, :, :])
                nc.gpsimd.dma_start(wo_t[p][:], wo_d[sl, :, :])
            xr_t = [xr_p.tile([128, QCH], bf16, tag=f"xr{m}", name=f"xr{m}") for m in range(4)]
            for m in range(4):
                nc.gpsimd.dma_start(xr_t[m][:], xres_d[m * 128:(m + 1) * 128, :])

            for _rep in range(reps):
                # ================= GroupNorm statistics =================
                x8_t = [xp.tile([128, N], f8, tag=f"x{p}", name=f"x{p}")
                        for p in range(4)]
                for p in range(4):
                    nc.sync.dma_start(x8_t[p][:], x8_d[p * 128:(p + 1) * 128, :])
                st2 = sm.tile([128, 8], f32, tag="st2", name="st2")
                for p in range(4):
                    nc.vector.tensor_reduce(
                        st2[:, 2 * p:2 * p + 1], x8_t[p][:],
                        mybir.AxisListType.X, ADD,
                    )
                    scr_v = scr_p.tile([128, N], f8, tag="scrv", name="scrv")
                    nc.scalar.activation(
                        scr_v[:], x8_t[p][:],
                        mybir.ActivationFunctionType.Square,
                        accum_out=st2[:, 2 * p + 1:2 * p + 2],
                    )
                # group stats: [Sx, Sxx] per channel -> per group (16 ch)
                pg_ps = ps.tile([8, 8], f32, tag="pp", name="pg")
                nc.tensor.matmul(pg_ps[:], g16_t[:], st2[:], start=True, stop=True)
                pg = sm.tile([8, 8], f32, tag="pg", name="pg_sb")
                nc.vector.tensor_copy(pg[:], pg_ps[:])
                # mu = pg[:, 0::2], E2 = pg[:, 1::2]  (both scaled by 1/GELEMS)
                mu = pg[:].rearrange("p (f two) -> p f two", two=2)[:, :, 0:1].squeeze(2)
                e2 = pg[:].rearrange("p (f two) -> p f two", two=2)[:, :, 1:2].squeeze(2)
                var = sm.tile([8, 4], f32, tag="var", name="var")
                nc.vector.tensor_tensor(var[:], mu, mu, MUL)
                nc.vector.tensor_sub(var[:], e2, var[:])
                nc.vector.tensor_scalar_add(var[:], var[:], EPS)
                sd = sm.tile([8, 4], f32, tag="sd", name="sd")
                nc.scalar.activation(sd[:], var[:],
                                     mybir.ActivationFunctionType.Sqrt)
                rb = sm.tile([8, 8], f32, tag="rb", name="rb")
                rstd = rb[:].rearrange("p (f two) -> p f two", two=2)[:, :, 0:1].squeeze(2)
                nmr = rb[:].rearrange("p (f two) -> p f two", two=2)[:, :, 1:2].squeeze(2)
                nc.vector.reciprocal(rstd, sd[:])
                nc.vector.tensor_tensor(nmr, mu, rstd, MUL)
                nc.vector.tensor_scalar_mul(nmr, nmr, -1.0)
                pc_ps = ps.tile([128, 8], f32, tag="pp", name="pc")
                nc.tensor.matmul(pc_ps[:], e16_t[:], rb[:], start=True, stop=True)
                pc = sm.tile([128, 8], f32, tag="pc", name="pc_sb")
                nc.vector.tensor_copy(pc[:], pc_ps[:])
                pc_r = pc[:].rearrange("p (f two) -> p f two", two=2)
                scale_a = sm.tile([128, 4], f32, tag="scale", name="scale")
                shift_a = sm.tile([128, 4], f32, tag="shift", name="shift")
                nc.vector.tensor_tensor(scale_a[:], gam_t[:], pc_r[:, :, 0:1].squeeze(2), MUL)
                nc.vector.tensor_tensor(shift_a[:], gam_t[:], pc_r[:, :, 1:2].squeeze(2), MUL)
                nc.vector.tensor_add(shift_a[:], shift_a[:], bet_t[:])

                # ================= normalize -> fp8 pairs =================
                xnp = [xnp_p.tile([128, 2, N], f8, tag=f"xn{pp}", name=f"xn{pp}")
                       for pp in range(2)]
                for h in range(2):
                    hsl = slice(h * 2048, (h + 1) * 2048)
                    for p in range(4):
                        if p < 2:
                            nc.gpsimd.tensor_scalar(
                                xnp[p // 2][:, p % 2, hsl], x8_t[p][:, hsl],
                                scale_a[:, p:p + 1], shift_a[:, p:p + 1], MUL, ADD,
                            )
                        elif p == 2:
                            nc.vector.tensor_scalar(
                                xnp[p // 2][:, p % 2, hsl], x8_t[p][:, hsl],
                                scale_a[:, p:p + 1], shift_a[:, p:p + 1], MUL, ADD,
                            )
                        else:
                            nc.scalar.activation(
                                xnp[p // 2][:, p % 2, hsl], x8_t[p][:, hsl],
                                mybir.ActivationFunctionType.Identity,
                                bias=shift_a[:, p:p + 1], scale=scale_a[:, p:p + 1],
                            )

                # ================= projections =================
                kp = [kp_p.tile([128, 2, N], f8, tag=f"k{pp}", name=f"k{pp}")
                      for pp in range(2)]
                vtp = [vp_p.tile([128, 2, 512], f8, tag=f"v{t}", name=f"v{t}")
                       for t in range(16)]
                qp = [qp_p.tile([128, 2, QCH], f8, tag=f"q{pp}", name=f"q{pp}")
                      for pp in range(2)]

                SSC = 1.0 / np.sqrt(C)  # softmax scale
                ci_state = {}

                def attn_begin(ci):
                    pv_ps = [pvp.tile([128, 512], f32, tag="pv", name="pv_ps")
                             for _ in range(4)]
                    rs_ps = rsp.tile([16, 512], f32, tag="rs", name="rs_ps")
                    ci_state[ci] = (pv_ps, rs_ps, [None])

                def attn_pair(ci, t):
                    pv_ps, rs_ps, ptt_box = ci_state[ci]
                    isl = slice(ci * 512, (ci + 1) * 512)
                    ptt = pt_p.tile([128, 2, 512], f8, tag="pt", name="pt")
                    for sub in range(2):
                        jt = 2 * t + sub
                        st_ps = ps.tile([128, 512], f32, tag="pp", name="st")
                        for pp in range(2):
                            nc.tensor.matmul(
                                st_ps[:],
                                kp[pp][:, :, jt * 128:(jt + 1) * 128],
                                qp[pp][:, :, isl],
                                start=(pp == 0), stop=(pp == 1), perf_mode=DR,
                            )
                        nc.scalar.activation(
                            ptt[:, sub, :], st_ps[:],
                            mybir.ActivationFunctionType.Exp,
                            bias=neg3[:], scale=SSC,
                        )
                    nc.tensor.matmul(
                        rs_ps[:], ones16[:], ptt[:],
                        start=(t == 0), stop=(t == 15), perf_mode=DR,
                    )
                    for m in range(4):
                        nc.tensor.matmul(
                            pv_ps[m][:],
                            vtp[t][:, :, m * 128:(m + 1) * 128],
                            ptt[:],
                            start=(t == 0), stop=(t == 15), perf_mode=DR,
                        )

                def attn_end(ci):
                    pv_ps, rs_ps, _ = ci_state[ci]
                    isl = slice(ci * 512, (ci + 1) * 512)
                    recip = sm.tile([1, 512], f32, tag=f"recip{ci}", name="recip")
                    nc.vector.reciprocal(recip[:], rs_ps[0:1, :])
                    recip_bc = sm.tile([128, 512], f32, tag=f"recip_bc{ci}",
                                       name="recip_bc")
                    nc.gpsimd.partition_broadcast(recip_bc[:], recip[:])
                    attp = [at_p.tile([128, 2, 512], f8, tag="att", name="att")
                            for _ in range(2)]
                    for m in range(4):
                        nc.vector.tensor_tensor(
                            attp[m // 2][:, m % 2, :], pv_ps[m][:], recip_bc[:], MUL,
                        )
                    for m in range(4):
                        po = ps.tile([128, 512], f32, tag="pp", name="po")
                        for pp in range(2):
                            nc.tensor.matmul(
                                po[:], wo_t[pp][:, :, m * 128:(m + 1) * 128],
                                attp[pp][:],
                                start=(pp == 0), stop=(pp == 1), perf_mode=DR,
                            )
                        yt = y_p.tile([128, 512], bf16, tag="y", name="yt")
                        nc.vector.scalar_tensor_tensor(
                            yt[:], po[:], 1.0 / WPRE, xr_t[m][:, isl], MUL, ADD,
                        )
                        nc.sync.dma_start(y_d[m * 128:(m + 1) * 128, isl], yt[:])

                # proj chunk jb emits K/V (all 8 chunks) and Q (jb<2); attention
                # ci0 pairs weave in once their kp columns exist (pair t needs
                # jt=2t+1 < 4*jb, i.e. t <= 2*jb - 1)
                attn_begin(0)
                emitted = 0
                for jb in range(8):
                    jsl = slice(jb * 512, (jb + 1) * 512)
                    # K: out[c_out m, j]
                    for m in range(4):
                        pk = ps.tile([128, 512], f32, tag="pp", name="pk")
                        for pp in range(2):
                            nc.tensor.matmul(
                                pk[:], wk_t[pp][:, :, m * 128:(m + 1) * 128],
                                xnp[pp][:, :, jsl],
                                start=(pp == 0), stop=(pp == 1), perf_mode=DR,
                            )
                        if m % 2 == 0:
                            nc.scalar.activation(
                                kp[m // 2][:, m % 2, jsl], pk[:],
                                mybir.ActivationFunctionType.Identity,
                                bias=bkp_t[:, m:m + 1], scale=1.0 / WPRE,
                            )
                        else:
                            nc.vector.tensor_scalar(
                                kp[m // 2][:, m % 2, jsl], pk[:],
                                1.0 / WPRE, bkp_t[:, m:m + 1], MUL, ADD,
                            )
                    # V^T: out[j, c_out] per jt
                    for jt4 in range(4):
                        jt = jb * 4 + jt4
                        pv_ = ps.tile([128, 512], f32, tag="pp", name="pvt")
                        for pp in range(2):
                            nc.tensor.matmul(
                                pv_[:], xnp[pp][:, :, jt * 128:(jt + 1) * 128],
                                wv_t[pp][:],
                                start=(pp == 0), stop=(pp == 1), perf_mode=DR,
                            )
                        if jt4 % 2 == 0:
                            nc.scalar.activation(
                                vtp[jt // 2][:, jt % 2, :], pv_[:],
                                mybir.ActivationFunctionType.Identity,
                                bias=zero_c[:], scale=1.0 / WPRE,
                            )
                        else:
                            nc.vector.tensor_scalar(
                                vtp[jt // 2][:, jt % 2, :], pv_[:],
                                1.0 / WPRE, None, MUL,
                            )
                    # Q (first two chunks only)
                    if jb < 2:
                        for m in range(4):
                            pq = ps.tile([128, 512], f32, tag="pp", name="pq")
                            for pp in range(2):
                                nc.tensor.matmul(
                                    pq[:], wq_t[pp][:, :, m * 128:(m + 1) * 128],
                                    xnp[pp][:, :, jsl],
                                    start=(pp == 0), stop=(pp == 1), perf_mode=DR,
                                )
                            if m % 2 == 0:
                                nc.scalar.activation(
                                    qp[m // 2][:, m % 2, jsl], pq[:],
                                    mybir.ActivationFunctionType.Identity,
                                    bias=bqp_t[:, m:m + 1], scale=1.0 / WPRE,
                                )
                            else:
                                nc.vector.tensor_scalar(
                                    qp[m // 2][:, m % 2, jsl], pq[:],
                                    1.0 / WPRE, bqp_t[:, m:m + 1], MUL, ADD,
                                )
                    # weave in ready ci0 attention pairs
                    if jb >= 2:
                        avail = min(2 * jb - 1, 16)
                        while emitted < avail:
                            attn_pair(0, emitted)
                            emitted += 1
                while emitted < 16:
                    attn_pair(0, emitted)
                    emitted += 1
                attn_end(0)
                attn_begin(1)
                for t in range(16):
                    attn_pair(1, t)
                attn_end(1)

    nc.compile()
    return nc


def get_nc(reps=1):
    if reps not in _NC_CACHE:
        _NC_CACHE[reps] = _build_nc(reps)
    return _NC_CACHE[reps]


def _pack_weight(w, prescale):
    # w: [c_out, c_in] -> wT [c_in, c_out] -> [pp*128+p, t, c_out]
    wT = np.ascontiguousarray(np.asarray(w, np.float32).T) * prescale
    arr = wT.reshape(2, 2, 128, C).transpose(0, 2, 1, 3).reshape(256, 2, C)
    return np.ascontiguousarray(arr).astype(E4)


def make_in_maps(x, gn_gamma, gn_beta, wq, bq, wk, bk, wv, bv, wo, bo):
    shared = {
        "wk8": _pack_weight(wk, WPRE),
        "wv8": _pack_weight(wv, WPRE),
        "wq8": _pack_weight(wq, WPRE),
        "wo8": _pack_weight(wo, WPRE),
        "bkp": np.ascontiguousarray(np.asarray(bk, np.float32).reshape(4, 128).T),
        "bqp": np.ascontiguousarray(np.asarray(bq, np.float32).reshape(4, 128).T),

        "gam": np.ascontiguousarray(np.asarray(gn_gamma, np.float32).reshape(4, 128).T),
        "bet": np.ascontiguousarray(np.asarray(gn_beta, np.float32).reshape(4, 128).T),
    }
    g16 = np.zeros((128, 8), np.float32)
    for i in range(128):
        g16[i, i // GSIZE] = 1.0 / GELEMS
    e16 = np.zeros((8, 128), np.float32)
    for i in range(128):
        e16[i // GSIZE, i] = 1.0
    shared["g16"] = g16
    shared["e16"] = e16

    bo_full = (np.asarray(bo, np.float32)
               + np.asarray(wo, np.float32) @ np.asarray(bv, np.float32))
    xf = np.asarray(x, np.float32).reshape(2, C, N)
    in_maps = []
    for cid in range(8):
        bi, qc = cid // 4, cid % 4
        xr = np.ascontiguousarray(np.roll(xf[bi], -qc * QCH, axis=1))
        in_maps.append({
            "x8": xr.astype(E4),
            "xres": (xr[:, :QCH] + bo_full[:, None]).astype(BF16),
            **shared,
        })
    return in_maps


def kernel(**inputs):
    from concourse.bass_utils import run_bass_kernel_spmd

    x = np.asarray(inputs["x"], np.float32)
    in_maps = make_in_maps(
        x, inputs["gn_gamma"], inputs["gn_beta"],
        inputs["wq"], inputs["bq"], inputs["wk"], inputs["bk"],
        inputs["wv"], inputs["bv"], inputs["wo"], inputs["bo"],
    )
    nc = get_nc(reps=1)
    res = run_bass_kernel_spmd(nc, in_maps, core_ids=list(range(8)), trace=False)
    out = np.empty((2, C, N), np.float32)
    for cid in range(8):
        bi, qc = cid // 4, cid % 4
        out[bi][:, qc * QCH:(qc + 1) * QCH] = np.asarray(
            res.results[cid]["y"]).astype(np.float32)
    return out.reshape(2, C, 64, 64)


if __name__ == "__main__":
    rng = np.random.default_rng(0)
    inputs = {
        "x": rng.standard_normal((2, C, 64, 64), dtype=np.float32),
        "gn_gamma": np.ones(C, np.float32),
        "gn_beta": np.zeros(C, np.float32),
    }
    s = 1.0 / np.sqrt(C)
    for nm in ("q", "k", "v", "o"):
        inputs[f"w{nm}"] = (rng.standard_normal((C, C), dtype=np.float32) * s)
        inputs[f"b{nm}"] = (rng.standard_normal(C, dtype=np.float32) * 0.01)
    out = kernel(**inputs)
    print("kernel ran, out shape", out.shape, "mean", out.mean())
